# revision 32
# baseline (speedup 1.0000x reference)
"""GQA kernel for Trainium2, 8 NeuronCores.

Problem: B=2, T=2048, D=2048, 16 query heads / 2 KV heads, d_head=128, causal.

Sharding: core c -> batch b = c//4, head-quarter q = c%4 (query heads
4q..4q+3, kv head q//2). Each core computes its 4 heads' attention and a
partial output projection (its Wo rows); host sums the 4 partials per batch
and adds bo. Partials are written bf16 (halves output DMA; host sums f32).

On-core dataflow (bf16 matmuls, fp32 PSUM accum), 4 rounds over 512-wide
tq-slices. Per round j: Q proj for slice j upfront (plus K proj round 0);
then per head an S phase: S^T tiles [tk,tq] -> exp (ACT) -> causal mask of
the 128-wide boundary block (GpSimd) -> row-sum accumulation in a bf16 racc
(DVE, 2-byte fast path). Because ACT exp (~620ns/tile) is slower than the
two PE matmuls per tile (~430ns), S matmuls are interleaved with "filler"
PE work pulled from a FIFO of thunks: K proj (j>0), V proj + PE transpose,
the previous round's output projection, the previous head's PV chain, and
row-sum matmuls (ones_bf16 @ racc_bf16, 1 cyc/row). Diagonal-band tiles are
trimmed to their causal width (512-128r), shrinking S/PV/exp/add/mask work.
This keeps the PE array continuously busy (p-state stays at max clock).
"""

import numpy as np
import ml_dtypes
from contextlib import ExitStack
from collections import deque

import concourse.bass as bass
from concourse import bacc
import concourse.mybir as mybir
import concourse.tile as tile
from concourse.bass_utils import run_bass_kernel_spmd
from concourse.masks import make_identity

F32 = mybir.dt.float32
BF16 = mybir.dt.bfloat16
IDENT = mybir.ActivationFunctionType.Identity
EXP = mybir.ActivationFunctionType.Exp

D = 2048
T = 2048
DH = 128
B = 2
HPC = 4            # query heads per core
NCORES = 8
SCALE = 1.0 / float(np.sqrt(128.0))

_CACHE = {}


def _build_nc():
    nc = bacc.Bacc("TRN2", target_bir_lowering=False, debug=False,
                   num_devices=NCORES)

    # host-marshalled layouts: partition-major [p, block, cols]
    xt0 = nc.dram_tensor("xt0", [128, 16 * 512], BF16, kind="ExternalInput")
    xt1 = nc.dram_tensor("xt1", [128, 16 * 512], BF16, kind="ExternalInput")
    xt2 = nc.dram_tensor("xt2", [128, 16 * 512], BF16, kind="ExternalInput")
    xt3 = nc.dram_tensor("xt3", [128, 16 * 512], BF16, kind="ExternalInput")
    # wq: [p, h(4), kb(16), 128]
    wq = nc.dram_tensor("wq", [128, HPC * 16 * 128], BF16, kind="ExternalInput")
    wk = nc.dram_tensor("wk", [128, 16 * 128], BF16, kind="ExternalInput")
    wv = nc.dram_tensor("wv", [128, 16 * 128], BF16, kind="ExternalInput")
    wo = nc.dram_tensor("wo", [128, HPC * D], BF16, kind="ExternalInput")
    bqm = nc.dram_tensor("bqm", [DH, HPC], F32, kind="ExternalInput")
    bkm = nc.dram_tensor("bkm", [DH, 1], F32, kind="ExternalInput")
    bvm = nc.dram_tensor("bvm", [DH, 1], F32, kind="ExternalInput")
    part = nc.dram_tensor("part", [T, D], BF16, kind="ExternalOutput")

    with ExitStack() as ctx:
        tc = ctx.enter_context(tile.TileContext(nc))
        persist = ctx.enter_context(tc.tile_pool(name="persist", bufs=1))
        work = ctx.enter_context(tc.tile_pool(name="work", bufs=3))
        psum = ctx.enter_context(tc.tile_pool(name="psum", bufs=2, space="PSUM"))

        # ---- SBUF destinations for inputs (big consolidated tiles) ----
        wk_sb = persist.tile([128, 16 * 128], BF16, tag="wk", name="wk_sb")
        wq_sb = persist.tile([128, HPC * 16 * 128], BF16, tag="wq",
                             name="wq_sb")
        wv_sb = persist.tile([128, 16 * 128], BF16, tag="wv", name="wv_sb")
        wo_sb = persist.tile([128, HPC * D], BF16, tag="wo", name="wo_sb")
        xs_sb = [persist.tile([128, 16 * 512], BF16, tag=f"x{j}",
                              name=f"x{j}_sb") for j in range(4)]
        bq_sb = persist.tile([DH, HPC], F32, tag="bq", name="bq_sb")
        bk_sb = persist.tile([DH, 1], F32, tag="bk", name="bk_sb")
        bv_sb = persist.tile([DH, 1], F32, tag="bv", name="bv_sb")

        def wkap(kb):
            return wk_sb[:, kb * 128:(kb + 1) * 128]

        def wvap(kb):
            return wv_sb[:, kb * 128:(kb + 1) * 128]

        def wqap(h, kb):
            o = h * 2048 + kb * 128
            return wq_sb[:, o:o + 128]

        def woap(h, nsl):
            return wo_sb[:, h * 2048 + nsl.start:h * 2048 + nsl.stop]

        def xap(kb, j):
            return xs_sb[j][:, kb * 512:(kb + 1) * 512]

        # ---- input DMAs first ----
        # balanced across the three queues (~120 GB/s each) so each
        # kb-group's (wk, x0, wq) chunks land together in consumption order
        def wqg(g):
            return slice(g * 2048, (g + 1) * 2048)

        def x0c(c):
            return slice(c * 2048, (c + 1) * 2048)

        xts = [xt0, xt1, xt2, xt3]
        # round-0 prefix (K + Q0) needs only wk + x0 + wq_h0 = 3MB;
        # later heads' wq stream in behind while S phases run
        nc.sync.dma_start(out=wk_sb[:, 0:1024], in_=wk[:, 0:1024])
        nc.scalar.dma_start(out=xs_sb[0][:, 0:2048], in_=xt0[:, 0:2048])
        nc.gpsimd.dma_start(out=wq_sb[:, 0:1024], in_=wq[:, 0:1024])
        nc.sync.dma_start(out=wk_sb[:, 1024:2048], in_=wk[:, 1024:2048])
        nc.scalar.dma_start(out=xs_sb[0][:, 2048:4096], in_=xt0[:, 2048:4096])
        nc.gpsimd.dma_start(out=wq_sb[:, 1024:2048], in_=wq[:, 1024:2048])
        nc.sync.dma_start(out=xs_sb[0][:, 4096:6144], in_=xt0[:, 4096:6144])
        nc.scalar.dma_start(out=xs_sb[0][:, 6144:8192], in_=xt0[:, 6144:8192])
        nc.gpsimd.dma_start(out=bq_sb, in_=bqm[:, :])
        nc.gpsimd.dma_start(out=bk_sb, in_=bkm[:, :])
        nc.gpsimd.dma_start(out=bv_sb, in_=bvm[:, :])
        nc.scalar.dma_start(out=wq_sb[:, 2048:4096], in_=wq[:, 2048:4096])
        nc.gpsimd.dma_start(out=wq_sb[:, 4096:6144], in_=wq[:, 4096:6144])
        nc.sync.dma_start(out=wv_sb, in_=wv[:, :])
        nc.scalar.dma_start(out=wq_sb[:, 6144:8192], in_=wq[:, 6144:8192])
        nc.gpsimd.dma_start(out=wo_sb, in_=wo[:, :])
        for j in range(1, 4):
            nc.sync.dma_start(out=xs_sb[j][:, 0:4096], in_=xts[j][:, 0:4096])
            nc.scalar.dma_start(out=xs_sb[j][:, 4096:8192],
                                in_=xts[j][:, 4096:8192])

        # ---- constants ----
        ones_bf = persist.tile([128, 128], BF16, tag="ones", name="ones_bf")
        nc.vector.memset(ones_bf, 1.0)
        ident = persist.tile([128, 128], BF16, tag="ident", name="ident")
        make_identity(nc, ident)

        # ---- persistent activations ----
        qT = [persist.tile([128, T], BF16, tag=f"qT{h}", name=f"qT{h}")
              for h in range(HPC)]
        kT = persist.tile([128, T], BF16, tag="kT", name="kT")
        v_sb = [persist.tile([128, DH], BF16, tag=f"v{t}", name=f"v{t}")
                for t in range(16)]
        oT = [persist.tile([128, T], BF16, tag=f"oT{h}", name=f"oT{h}")
              for h in range(HPC)]

        # ---- filler machinery: FIFO of (kind, fn) emitting one PE op each ----
        filler = deque()

        def pull(n):
            for _ in range(n):
                if not filler:
                    return
                filler.popleft()[1]()

        def drain(kinds=None):
            while filler and (kinds is None or filler[0][0] in kinds):
                filler.popleft()[1]()

        def push_Q(h, j):
            # next head's Q projection, pushed to the FRONT so it drains
            # during the current head's S phase (filler)
            sl = slice(512 * j, 512 * (j + 1))
            st = {}

            def mk(kb):
                def f():
                    if kb == 0:
                        st['p'] = psum.tile([128, 512], F32, tag="fill",
                                            bufs=2, name=f"qf{j}_{h}")
                    nc.tensor.matmul(out=st['p'], lhsT=wqap(h, kb),
                                     rhs=xap(kb, j),
                                     start=(kb == 0), stop=(kb == 15))
                    if kb == 15:
                        nc.scalar.activation(out=qT[h][:, sl], in_=st['p'],
                                             func=IDENT,
                                             bias=bq_sb[:, h:h + 1],
                                             scale=1.0)
                return f
            for kb in reversed(range(16)):
                filler.appendleft(('Qp', mk(kb)))

        def push_V(j):
            st = {}

            def mk(kb):
                def f():
                    if kb == 0:
                        st['p'] = psum.tile([128, 512], F32, tag="fill",
                                            bufs=2, name=f"vps{j}")
                    nc.tensor.matmul(out=st['p'], lhsT=wvap(kb),
                                     rhs=xap(kb, j),
                                     start=(kb == 0), stop=(kb == 15))
                    if kb == 15:
                        st['vt'] = work.tile([128, 512], BF16, tag="vt",
                                             bufs=2, name=f"vt{j}")
                        nc.scalar.activation(out=st['vt'], in_=st['p'],
                                             func=IDENT, bias=bv_sb[:, 0:1],
                                             scale=1.0)
                return f
            for kb in range(16):
                filler.append(('V', mk(kb)))

            def mt(sub):
                def f():
                    if sub == 0:
                        st['tp'] = psum.tile([128, 512], BF16, tag="fill",
                                             bufs=2, name=f"vtp{j}")
                    nc.tensor.transpose(st['tp'][:, sub * 128:(sub + 1) * 128],
                                        st['vt'][:, sub * 128:(sub + 1) * 128],
                                        ident)
                    nc.vector.tensor_copy(
                        out=v_sb[4 * j + sub],
                        in_=st['tp'][:, sub * 128:(sub + 1) * 128])
                return f
            for sub in range(4):
                filler.append(('V', mt(sub)))

        def push_O(j):
            # output projection for the 4 t-tiles of tq-slice j
            for sub in range(4):
                tt = 4 * j + sub
                st = {}

                def mk(n, h, tt=tt, st=st):
                    def f():
                        if h == 0:
                            if n == 0:
                                st['g'] = work.tile([128, D], BF16, tag="ostg",
                                                    bufs=2, name=f"ostg{tt}")
                            st['p'] = psum.tile([128, 512], F32, tag="fill",
                                                bufs=2, name=f"ops{tt}_{n}")
                        nsl = slice(n * 512, (n + 1) * 512)
                        nc.tensor.matmul(
                            out=st['p'],
                            lhsT=oT[h][:, tt * 128:(tt + 1) * 128],
                            rhs=woap(h, nsl),
                            start=(h == 0), stop=(h == HPC - 1))
                        if h == HPC - 1:
                            if n % 2 == 0:
                                nc.scalar.activation(out=st['g'][:, nsl],
                                                     in_=st['p'], func=IDENT,
                                                     bias=0.0, scale=1.0)
                            else:
                                nc.vector.tensor_copy(out=st['g'][:, nsl],
                                                      in_=st['p'])
                            nc.sync.dma_start(
                                out=part[tt * 128:(tt + 1) * 128, nsl],
                                in_=st['g'][:, nsl])
                    return f
                for n in range(4):
                    for h in range(HPC):
                        filler.append(('O', mk(n, h)))

        def push_PV(h, j, pts, racc):
            ntk = 4 * (j + 1)
            st = {}

            def mk(tkb):
                r = tkb - 4 * j
                off = 128 * r if r > 0 else 0

                def f():
                    if tkb == 0:
                        st['p'] = psum.tile([128, 512], F32, tag="att",
                                            bufs=3, name=f"otps{h}_{j}")
                    nc.tensor.matmul(out=st['p'][:, off:512],
                                     lhsT=v_sb[tkb], rhs=pts[tkb][:, off:512],
                                     start=(tkb == 0), stop=(tkb == ntk - 1))
                return f
            for tkb in range(ntk):
                filler.append(('PV', mk(tkb)))

            def rs():
                rsb = psum.tile([128, 512], F32, tag="att", bufs=3,
                                name=f"rsb{h}_{j}")
                nc.tensor.matmul(out=rsb, lhsT=ones_bf, rhs=racc,
                                 start=True, stop=True)
                rinv = work.tile([128, 512], F32, tag="rinv", bufs=2,
                                 name=f"rinv{h}_{j}")
                nc.vector.reciprocal_approx_fast(rinv, rsb)
                nc.vector.tensor_mul(out=oT[h][:, 512 * j:512 * (j + 1)],
                                     in0=st['p'], in1=rinv)
            filler.append(('RS', rs))

        # ================= main schedule =================
        for j in range(4):
            sl = slice(512 * j, 512 * (j + 1))

            # --- prefix: K proj + Q proj of head 0, interleaved per kb ---
            kps = psum.tile([128, 512], F32, tag="sp", bufs=3,
                            name=f"kps{j}")
            qps0 = psum.tile([128, 512], F32, tag="sp", bufs=3,
                             name=f"qps{j}_0")
            for kb in range(16):
                nc.tensor.matmul(out=kps, lhsT=wkap(kb), rhs=xap(kb, j),
                                 start=(kb == 0), stop=(kb == 15))
                nc.tensor.matmul(out=qps0, lhsT=wqap(0, kb), rhs=xap(kb, j),
                                 start=(kb == 0), stop=(kb == 15))
            nc.scalar.activation(out=kT[:, sl], in_=kps, func=IDENT,
                                 bias=bk_sb[:, 0:1], scale=1.0)
            nc.scalar.activation(out=qT[0][:, sl], in_=qps0, func=IDENT,
                                 bias=bq_sb[:, 0:1], scale=1.0)

            # --- queue this round's filler ---
            push_V(j)
            if j > 0:
                push_O(j - 1)

            # --- attention S phases ---
            ntk = 4 * (j + 1)
            for h in range(HPC):
                if h > 0:
                    drain(kinds=('Qp',))   # qT[h] must be complete
                if h < 3:
                    push_Q(h + 1, j)
                racc = work.tile([128, 512], BF16, tag="racc", bufs=3,
                                 name=f"racc{h}_{j}")
                pts = []
                for tkb in range(ntk):
                    r = tkb - 4 * j
                    off = 128 * r if r > 0 else 0
                    sps = psum.tile([128, 512], F32, tag="sp", bufs=3,
                                    name=f"sps{h}_{j}_{tkb}")
                    nc.tensor.matmul(
                        out=sps[:, off:512],
                        lhsT=kT[:, tkb * 128:(tkb + 1) * 128],
                        rhs=qT[h][:, 512 * j + off:512 * (j + 1)],
                        start=True, stop=True)
                    pt = work.tile([128, 512], BF16, tag="pt", bufs=28,
                                   name=f"pt{h}_{j}_{tkb}")
                    nc.scalar.activation(out=pt[:, off:512],
                                         in_=sps[:, off:512],
                                         func=EXP, scale=SCALE)
                    if r >= 0:
                        nc.gpsimd.affine_select(
                            out=pt[:, off:off + 128], in_=pt[:, off:off + 128],
                            compare_op=mybir.AluOpType.is_ge,
                            fill=0.0, base=0,
                            pattern=[[1, 128]],
                            channel_multiplier=-1)
                    if tkb == 0:
                        nc.vector.tensor_copy(out=racc, in_=pt)
                    else:
                        nc.vector.tensor_add(out=racc[:, off:512],
                                             in0=racc[:, off:512],
                                             in1=pt[:, off:512])
                    pts.append(pt)
                    pull(2 if r < 2 else 1)
                push_PV(h, j, pts, racc)

        # tail: previous-head PV/rowsum + O proj of slice 3
        drain()
        push_O(3)
        drain()

    nc.compile()
    return nc


def _get_nc():
    if "nc" not in _CACHE:
        _CACHE["nc"] = _build_nc()
    return _CACHE["nc"]


def _bf16(a):
    return np.ascontiguousarray(a.astype(ml_dtypes.bfloat16))


def kernel(x, Wq, bq, Wk, bk, Wv, bv, Wo, bo, **kw):
    x = np.asarray(x, dtype=np.float32)
    Wq = np.asarray(Wq, dtype=np.float32)
    Wk = np.asarray(Wk, dtype=np.float32)
    Wv = np.asarray(Wv, dtype=np.float32)
    Wo = np.asarray(Wo, dtype=np.float32)
    bq = np.asarray(bq, dtype=np.float32)
    bk = np.asarray(bk, dtype=np.float32)
    bv = np.asarray(bv, dtype=np.float32)
    bo = np.asarray(bo, dtype=np.float32)

    nc = _get_nc()

    def pmaj(a, nblk, cols):
        # [nblk*128, cols] -> partition-major [128, nblk*cols]
        return np.ascontiguousarray(
            a.reshape(nblk, 128, cols).transpose(1, 0, 2).reshape(
                128, nblk * cols))

    xt_b = []
    for b in range(B):
        xb = x[b].T                                 # [D, T]
        xt_b.append([_bf16(pmaj(xb[:, 512 * j:512 * (j + 1)], 16, 512))
                     for j in range(4)])
    kv_cache = {}
    for kv in (0, 1):
        kv_cache[kv] = (_bf16(pmaj(Wk[:, kv * DH:(kv + 1) * DH], 16, DH)),
                        _bf16(pmaj(Wv[:, kv * DH:(kv + 1) * DH], 16, DH)))
    in_maps = []
    for c in range(NCORES):
        b = c // 4
        q = c % 4
        hs = q * HPC * DH          # column start in Wq / row start in Wo
        kv = q // 2
        # wq: [p, h, kb, 128] layout
        wq_m = np.ascontiguousarray(
            Wq[:, hs:hs + HPC * DH].reshape(16, 128, HPC, DH)
            .transpose(1, 2, 0, 3).reshape(128, HPC * 16 * DH))
        wo_m = pmaj(Wo[hs:hs + HPC * DH, :], HPC, D)
        bq_m = np.ascontiguousarray(
            bq[hs:hs + HPC * DH].reshape(HPC, DH).T)          # [128, 4]
        bk_m = np.ascontiguousarray(
            bk[kv * DH:(kv + 1) * DH].reshape(DH, 1))         # [128, 1]
        bv_m = np.ascontiguousarray(
            bv[kv * DH:(kv + 1) * DH].reshape(DH, 1))         # [128, 1]
        in_maps.append({
            "xt0": xt_b[b][0],
            "xt1": xt_b[b][1],
            "xt2": xt_b[b][2],
            "xt3": xt_b[b][3],
            "wq": _bf16(wq_m),
            "wk": kv_cache[kv][0],
            "wv": kv_cache[kv][1],
            "wo": _bf16(wo_m),
            "bqm": bq_m,
            "bkm": bk_m,
            "bvm": bv_m,
        })

    res = run_bass_kernel_spmd(nc, in_maps, list(range(NCORES)),
                               **kw.get("_run_kwargs", {}))
    if kw.get("_return_res"):
        return res
    parts = [res.results[c]["part"] for c in range(NCORES)]
    out = np.empty((B, T, D), dtype=np.float32)
    for b in range(B):
        acc = parts[4 * b].astype(np.float32)
        for q in range(1, 4):
            acc = acc + parts[4 * b + q].astype(np.float32)
        out[b] = acc + bo[None, :]
    return out


# revision 33
# speedup vs baseline: 1.0571x; 1.0571x over previous
"""GQA kernel for Trainium2, 8 NeuronCores.

Problem: B=2, T=2048, D=2048, 16 query heads / 2 KV heads, d_head=128, causal.

Sharding: core c -> batch b = c//4, head-quarter q = c%4 (query heads
4q..4q+3, kv head q//2). Each core computes its 4 heads' attention and a
partial output projection (its Wo rows); host sums the 4 partials per batch
and adds bo. Partials are written bf16 (halves output DMA; host sums f32).

On-core dataflow (bf16 matmuls, fp32 PSUM accum), 4 rounds over 512-wide
tq-slices. Per round j: Q proj for slice j upfront (plus K proj round 0);
then per head an S phase: S^T tiles [tk,tq] -> exp (ACT) -> causal mask of
the 128-wide boundary block (GpSimd) -> row-sum accumulation in a bf16 racc
(DVE, 2-byte fast path). Because ACT exp (~620ns/tile) is slower than the
two PE matmuls per tile (~430ns), S matmuls are interleaved with "filler"
PE work pulled from a FIFO of thunks: K proj (j>0), V proj + PE transpose,
the previous round's output projection, the previous head's PV chain, and
row-sum matmuls (ones_bf16 @ racc_bf16, 1 cyc/row). Diagonal-band tiles are
trimmed to their causal width (512-128r), shrinking S/PV/exp/add/mask work.
This keeps the PE array continuously busy (p-state stays at max clock).
"""

import numpy as np
import ml_dtypes
from contextlib import ExitStack
from collections import deque

import concourse.bass as bass
from concourse import bacc
import concourse.mybir as mybir
import concourse.tile as tile
from concourse.bass_utils import run_bass_kernel_spmd
from concourse.masks import make_identity

F32 = mybir.dt.float32
BF16 = mybir.dt.bfloat16
IDENT = mybir.ActivationFunctionType.Identity
EXP = mybir.ActivationFunctionType.Exp

D = 2048
T = 2048
DH = 128
B = 2
HPC = 4            # query heads per core
NCORES = 8
SCALE = 1.0 / float(np.sqrt(128.0))

_CACHE = {}


def _build_nc():
    nc = bacc.Bacc("TRN2", target_bir_lowering=False, debug=False,
                   num_devices=NCORES)

    # host-marshalled layouts: partition-major [p, block, cols]
    xt0 = nc.dram_tensor("xt0", [128, 16 * 512], BF16, kind="ExternalInput")
    xt1 = nc.dram_tensor("xt1", [128, 16 * 512], BF16, kind="ExternalInput")
    xt2 = nc.dram_tensor("xt2", [128, 16 * 512], BF16, kind="ExternalInput")
    xt3 = nc.dram_tensor("xt3", [128, 16 * 512], BF16, kind="ExternalInput")
    # wq: [p, h(4), kb(16), 128]
    wq = nc.dram_tensor("wq", [128, HPC * 16 * 128], BF16, kind="ExternalInput")
    wk = nc.dram_tensor("wk", [128, 16 * 128], BF16, kind="ExternalInput")
    wv = nc.dram_tensor("wv", [128, 16 * 128], BF16, kind="ExternalInput")
    wo = nc.dram_tensor("wo", [128, HPC * D], BF16, kind="ExternalInput")
    bqm = nc.dram_tensor("bqm", [DH, HPC], F32, kind="ExternalInput")
    bkm = nc.dram_tensor("bkm", [DH, 1], F32, kind="ExternalInput")
    bvm = nc.dram_tensor("bvm", [DH, 1], F32, kind="ExternalInput")
    part = nc.dram_tensor("part", [T, D], BF16, kind="ExternalOutput")

    with ExitStack() as ctx:
        tc = ctx.enter_context(tile.TileContext(nc))
        persist = ctx.enter_context(tc.tile_pool(name="persist", bufs=1))
        work = ctx.enter_context(tc.tile_pool(name="work", bufs=3))
        psum = ctx.enter_context(tc.tile_pool(name="psum", bufs=2, space="PSUM"))

        # ---- SBUF destinations for inputs (big consolidated tiles) ----
        wk_sb = persist.tile([128, 16 * 128], BF16, tag="wk", name="wk_sb")
        wq_sb = persist.tile([128, HPC * 16 * 128], BF16, tag="wq",
                             name="wq_sb")
        wv_sb = persist.tile([128, 16 * 128], BF16, tag="wv", name="wv_sb")
        wo_sb = persist.tile([128, HPC * D], BF16, tag="wo", name="wo_sb")
        xs_sb = [persist.tile([128, 16 * 512], BF16, tag=f"x{j}",
                              name=f"x{j}_sb") for j in range(4)]
        bq_sb = persist.tile([DH, HPC], F32, tag="bq", name="bq_sb")
        bk_sb = persist.tile([DH, 1], F32, tag="bk", name="bk_sb")
        bv_sb = persist.tile([DH, 1], F32, tag="bv", name="bv_sb")

        def wkap(kb):
            return wk_sb[:, kb * 128:(kb + 1) * 128]

        def wvap(kb):
            return wv_sb[:, kb * 128:(kb + 1) * 128]

        def wqap(h, kb):
            o = h * 2048 + kb * 128
            return wq_sb[:, o:o + 128]

        def woap(h, nsl):
            return wo_sb[:, h * 2048 + nsl.start:h * 2048 + nsl.stop]

        def xap(kb, j):
            return xs_sb[j][:, kb * 512:(kb + 1) * 512]

        # ---- input DMAs first ----
        # balanced across the three queues (~120 GB/s each) so each
        # kb-group's (wk, x0, wq) chunks land together in consumption order
        def wqg(g):
            return slice(g * 2048, (g + 1) * 2048)

        def x0c(c):
            return slice(c * 2048, (c + 1) * 2048)

        xts = [xt0, xt1, xt2, xt3]
        # round-0 prefix (K + Q0) needs only wk + x0 + wq_h0 = 3MB;
        # later heads' wq stream in behind while S phases run
        nc.sync.dma_start(out=wk_sb[:, 0:1024], in_=wk[:, 0:1024])
        nc.scalar.dma_start(out=xs_sb[0][:, 0:2048], in_=xt0[:, 0:2048])
        nc.gpsimd.dma_start(out=wq_sb[:, 0:1024], in_=wq[:, 0:1024])
        nc.sync.dma_start(out=wk_sb[:, 1024:2048], in_=wk[:, 1024:2048])
        nc.scalar.dma_start(out=xs_sb[0][:, 2048:4096], in_=xt0[:, 2048:4096])
        nc.gpsimd.dma_start(out=wq_sb[:, 1024:2048], in_=wq[:, 1024:2048])
        nc.sync.dma_start(out=xs_sb[0][:, 4096:6144], in_=xt0[:, 4096:6144])
        nc.scalar.dma_start(out=xs_sb[0][:, 6144:8192], in_=xt0[:, 6144:8192])
        nc.gpsimd.dma_start(out=bq_sb, in_=bqm[:, :])
        nc.gpsimd.dma_start(out=bk_sb, in_=bkm[:, :])
        nc.gpsimd.dma_start(out=bv_sb, in_=bvm[:, :])
        nc.scalar.dma_start(out=wq_sb[:, 2048:4096], in_=wq[:, 2048:4096])
        nc.gpsimd.dma_start(out=wq_sb[:, 4096:6144], in_=wq[:, 4096:6144])
        nc.sync.dma_start(out=wv_sb, in_=wv[:, :])
        nc.scalar.dma_start(out=wq_sb[:, 6144:8192], in_=wq[:, 6144:8192])
        nc.gpsimd.dma_start(out=wo_sb, in_=wo[:, :])
        for j in range(1, 4):
            nc.sync.dma_start(out=xs_sb[j][:, 0:4096], in_=xts[j][:, 0:4096])
            nc.scalar.dma_start(out=xs_sb[j][:, 4096:8192],
                                in_=xts[j][:, 4096:8192])

        # ---- constants ----
        ones_bf = persist.tile([128, 128], BF16, tag="ones", name="ones_bf")
        nc.vector.memset(ones_bf, 1.0)
        ident = persist.tile([128, 128], BF16, tag="ident", name="ident")
        make_identity(nc, ident)

        # ---- persistent activations ----
        qT = [persist.tile([128, T], BF16, tag=f"qT{h}", name=f"qT{h}")
              for h in range(HPC)]
        kT = persist.tile([128, T], BF16, tag="kT", name="kT")
        v_sb = [persist.tile([128, DH], BF16, tag=f"v{t}", name=f"v{t}")
                for t in range(16)]
        oT = [persist.tile([128, T], BF16, tag=f"oT{h}", name=f"oT{h}")
              for h in range(HPC)]

        # ---- filler machinery: FIFO of (kind, fn) emitting one PE op each ----
        filler = deque()

        def pull(n):
            for _ in range(n):
                if not filler:
                    return
                filler.popleft()[1]()

        def drain(kinds=None):
            while filler and (kinds is None or filler[0][0] in kinds):
                filler.popleft()[1]()

        def push_Q(h, j):
            # next head's Q projection, pushed to the FRONT so it drains
            # during the current head's S phase (filler)
            sl = slice(512 * j, 512 * (j + 1))
            st = {}

            def mk(kb):
                def f():
                    if kb == 0:
                        st['p'] = psum.tile([128, 512], F32, tag="fill",
                                            bufs=2, name=f"qf{j}_{h}")
                    nc.tensor.matmul(out=st['p'], lhsT=wqap(h, kb),
                                     rhs=xap(kb, j),
                                     start=(kb == 0), stop=(kb == 15))
                    if kb == 15:
                        nc.scalar.activation(out=qT[h][:, sl], in_=st['p'],
                                             func=IDENT,
                                             bias=bq_sb[:, h:h + 1],
                                             scale=1.0)
                return f
            for kb in reversed(range(16)):
                filler.appendleft(('Qp', mk(kb)))

        def push_V(j):
            st = {}

            def mk(kb):
                def f():
                    if kb == 0:
                        st['p'] = psum.tile([128, 512], F32, tag="fill",
                                            bufs=2, name=f"vps{j}")
                    nc.tensor.matmul(out=st['p'], lhsT=wvap(kb),
                                     rhs=xap(kb, j),
                                     start=(kb == 0), stop=(kb == 15))
                    if kb == 15:
                        st['vt'] = work.tile([128, 512], BF16, tag="vt",
                                             bufs=2, name=f"vt{j}")
                        nc.scalar.activation(out=st['vt'], in_=st['p'],
                                             func=IDENT, bias=bv_sb[:, 0:1],
                                             scale=1.0)
                return f
            for kb in range(16):
                filler.append(('V', mk(kb)))

            def mt(sub):
                def f():
                    if sub == 0:
                        st['tp'] = psum.tile([128, 512], BF16, tag="fill",
                                             bufs=2, name=f"vtp{j}")
                    nc.tensor.transpose(st['tp'][:, sub * 128:(sub + 1) * 128],
                                        st['vt'][:, sub * 128:(sub + 1) * 128],
                                        ident)
                    nc.vector.tensor_copy(
                        out=v_sb[4 * j + sub],
                        in_=st['tp'][:, sub * 128:(sub + 1) * 128])
                return f
            for sub in range(4):
                filler.append(('V', mt(sub)))

        def push_O(j):
            # output projection for the 4 t-tiles of tq-slice j
            for sub in range(4):
                tt = 4 * j + sub
                st = {}

                def mk(n, h, tt=tt, st=st):
                    def f():
                        if h == 0:
                            if n == 0:
                                st['g'] = work.tile([128, D], BF16, tag="ostg",
                                                    bufs=2, name=f"ostg{tt}")
                            st['p'] = psum.tile([128, 512], F32, tag="fill",
                                                bufs=2, name=f"ops{tt}_{n}")
                        nsl = slice(n * 512, (n + 1) * 512)
                        nc.tensor.matmul(
                            out=st['p'],
                            lhsT=oT[h][:, tt * 128:(tt + 1) * 128],
                            rhs=woap(h, nsl),
                            start=(h == 0), stop=(h == HPC - 1))
                        if h == HPC - 1:
                            if n % 2 == 0:
                                nc.scalar.activation(out=st['g'][:, nsl],
                                                     in_=st['p'], func=IDENT,
                                                     bias=0.0, scale=1.0)
                            else:
                                nc.vector.tensor_copy(out=st['g'][:, nsl],
                                                      in_=st['p'])
                            nc.sync.dma_start(
                                out=part[tt * 128:(tt + 1) * 128, nsl],
                                in_=st['g'][:, nsl])
                    return f
                for n in range(4):
                    for h in range(HPC):
                        filler.append(('O', mk(n, h)))

        def push_PV(h, j, pts, racc):
            ntk = 4 * (j + 1)
            st = {}

            def mk(tkb):
                r = tkb - 4 * j
                off = 128 * r if r > 0 else 0

                def f():
                    if tkb == 0:
                        st['p'] = psum.tile([128, 512], F32, tag="att",
                                            bufs=3, name=f"otps{h}_{j}")
                    nc.tensor.matmul(out=st['p'][:, off:512],
                                     lhsT=v_sb[tkb], rhs=pts[tkb][:, off:512],
                                     start=(tkb == 0), stop=(tkb == ntk - 1))
                return f
            for tkb in range(ntk):
                filler.append(('PV', mk(tkb)))

            def rs():
                rsb = psum.tile([128, 512], F32, tag="att", bufs=3,
                                name=f"rsb{h}_{j}")
                nc.tensor.matmul(out=rsb, lhsT=ones_bf, rhs=racc,
                                 start=True, stop=True)
                rinv = work.tile([128, 512], F32, tag="rinv", bufs=2,
                                 name=f"rinv{h}_{j}")
                nc.vector.reciprocal_approx_fast(rinv, rsb)
                nc.vector.tensor_mul(out=oT[h][:, 512 * j:512 * (j + 1)],
                                     in0=st['p'], in1=rinv)
            filler.append(('RS', rs))

        # ================= main schedule =================
        for j in range(4):
            sl = slice(512 * j, 512 * (j + 1))

            drain()
            # --- prefix: K proj + Q proj of head 0, interleaved per kb ---
            kps = psum.tile([128, 512], F32, tag="sp", bufs=3,
                            name=f"kps{j}")
            qps0 = psum.tile([128, 512], F32, tag="sp", bufs=3,
                             name=f"qps{j}_0")
            for kb in range(16):
                nc.tensor.matmul(out=kps, lhsT=wkap(kb), rhs=xap(kb, j),
                                 start=(kb == 0), stop=(kb == 15))
                nc.tensor.matmul(out=qps0, lhsT=wqap(0, kb), rhs=xap(kb, j),
                                 start=(kb == 0), stop=(kb == 15))
            nc.scalar.activation(out=kT[:, sl], in_=kps, func=IDENT,
                                 bias=bk_sb[:, 0:1], scale=1.0)
            nc.scalar.activation(out=qT[0][:, sl], in_=qps0, func=IDENT,
                                 bias=bq_sb[:, 0:1], scale=1.0)

            # --- queue this round's filler ---
            push_V(j)
            if j > 0:
                push_O(j - 1)

            # --- attention S phases ---
            ntk = 4 * (j + 1)
            for h in range(HPC):
                if h > 0:
                    drain(kinds=('Qp',))   # qT[h] must be complete
                if h < 3:
                    push_Q(h + 1, j)
                racc = work.tile([128, 512], BF16, tag="racc", bufs=3,
                                 name=f"racc{h}_{j}")
                pts = []
                for tkb in range(ntk):
                    r = tkb - 4 * j
                    off = 128 * r if r > 0 else 0
                    sps = psum.tile([128, 512], F32, tag="sp", bufs=3,
                                    name=f"sps{h}_{j}_{tkb}")
                    nc.tensor.matmul(
                        out=sps[:, off:512],
                        lhsT=kT[:, tkb * 128:(tkb + 1) * 128],
                        rhs=qT[h][:, 512 * j + off:512 * (j + 1)],
                        start=True, stop=True)
                    pt = work.tile([128, 512], BF16, tag="pt", bufs=28,
                                   name=f"pt{h}_{j}_{tkb}")
                    nc.scalar.activation(out=pt[:, off:512],
                                         in_=sps[:, off:512],
                                         func=EXP, scale=SCALE)
                    if r >= 0:
                        nc.gpsimd.affine_select(
                            out=pt[:, off:off + 128], in_=pt[:, off:off + 128],
                            compare_op=mybir.AluOpType.is_ge,
                            fill=0.0, base=0,
                            pattern=[[1, 128]],
                            channel_multiplier=-1)
                    if tkb == 0:
                        nc.vector.tensor_copy(out=racc, in_=pt)
                    else:
                        nc.vector.tensor_add(out=racc[:, off:512],
                                             in0=racc[:, off:512],
                                             in1=pt[:, off:512])
                    pts.append(pt)
                    pull(2 if r < 2 else 1)
                push_PV(h, j, pts, racc)

        # tail: previous-head PV/rowsum + O proj of slice 3
        drain()
        push_O(3)
        drain()

    nc.compile()
    return nc


def _get_nc():
    if "nc" not in _CACHE:
        _CACHE["nc"] = _build_nc()
    return _CACHE["nc"]


def _bf16(a):
    return np.ascontiguousarray(a.astype(ml_dtypes.bfloat16))


def kernel(x, Wq, bq, Wk, bk, Wv, bv, Wo, bo, **kw):
    x = np.asarray(x, dtype=np.float32)
    Wq = np.asarray(Wq, dtype=np.float32)
    Wk = np.asarray(Wk, dtype=np.float32)
    Wv = np.asarray(Wv, dtype=np.float32)
    Wo = np.asarray(Wo, dtype=np.float32)
    bq = np.asarray(bq, dtype=np.float32)
    bk = np.asarray(bk, dtype=np.float32)
    bv = np.asarray(bv, dtype=np.float32)
    bo = np.asarray(bo, dtype=np.float32)

    nc = _get_nc()

    def pmaj(a, nblk, cols):
        # [nblk*128, cols] -> partition-major [128, nblk*cols]
        return np.ascontiguousarray(
            a.reshape(nblk, 128, cols).transpose(1, 0, 2).reshape(
                128, nblk * cols))

    xt_b = []
    for b in range(B):
        xb = x[b].T                                 # [D, T]
        xt_b.append([_bf16(pmaj(xb[:, 512 * j:512 * (j + 1)], 16, 512))
                     for j in range(4)])
    kv_cache = {}
    for kv in (0, 1):
        kv_cache[kv] = (_bf16(pmaj(Wk[:, kv * DH:(kv + 1) * DH], 16, DH)),
                        _bf16(pmaj(Wv[:, kv * DH:(kv + 1) * DH], 16, DH)))
    in_maps = []
    for c in range(NCORES):
        b = c // 4
        q = c % 4
        hs = q * HPC * DH          # column start in Wq / row start in Wo
        kv = q // 2
        # wq: [p, h, kb, 128] layout
        wq_m = np.ascontiguousarray(
            Wq[:, hs:hs + HPC * DH].reshape(16, 128, HPC, DH)
            .transpose(1, 2, 0, 3).reshape(128, HPC * 16 * DH))
        wo_m = pmaj(Wo[hs:hs + HPC * DH, :], HPC, D)
        bq_m = np.ascontiguousarray(
            bq[hs:hs + HPC * DH].reshape(HPC, DH).T)          # [128, 4]
        bk_m = np.ascontiguousarray(
            bk[kv * DH:(kv + 1) * DH].reshape(DH, 1))         # [128, 1]
        bv_m = np.ascontiguousarray(
            bv[kv * DH:(kv + 1) * DH].reshape(DH, 1))         # [128, 1]
        in_maps.append({
            "xt0": xt_b[b][0],
            "xt1": xt_b[b][1],
            "xt2": xt_b[b][2],
            "xt3": xt_b[b][3],
            "wq": _bf16(wq_m),
            "wk": kv_cache[kv][0],
            "wv": kv_cache[kv][1],
            "wo": _bf16(wo_m),
            "bqm": bq_m,
            "bkm": bk_m,
            "bvm": bv_m,
        })

    res = run_bass_kernel_spmd(nc, in_maps, list(range(NCORES)),
                               **kw.get("_run_kwargs", {}))
    if kw.get("_return_res"):
        return res
    parts = [res.results[c]["part"] for c in range(NCORES)]
    out = np.empty((B, T, D), dtype=np.float32)
    for b in range(B):
        acc = parts[4 * b].astype(np.float32)
        for q in range(1, 4):
            acc = acc + parts[4 * b + q].astype(np.float32)
        out[b] = acc + bo[None, :]
    return out


# revision 35
# speedup vs baseline: 1.0760x; 1.0178x over previous
"""GQA kernel for Trainium2, 8 NeuronCores.

Problem: B=2, T=2048, D=2048, 16 query heads / 2 KV heads, d_head=128, causal.

Sharding: core c -> batch b = c//4, head-quarter q = c%4 (query heads
4q..4q+3, kv head q//2). Each core computes its 4 heads' attention and a
partial output projection (its Wo rows); host sums the 4 partials per batch
and adds bo. Partials are written bf16 (halves output DMA; host sums f32).

On-core dataflow (bf16 matmuls, fp32 PSUM accum), 4 rounds over 512-wide
tq-slices. Per round j: Q proj for slice j upfront (plus K proj round 0);
then per head an S phase: S^T tiles [tk,tq] -> exp (ACT) -> causal mask of
the 128-wide boundary block (GpSimd) -> row-sum accumulation in a bf16 racc
(DVE, 2-byte fast path). Because ACT exp (~620ns/tile) is slower than the
two PE matmuls per tile (~430ns), S matmuls are interleaved with "filler"
PE work pulled from a FIFO of thunks: K proj (j>0), V proj + PE transpose,
the previous round's output projection, the previous head's PV chain, and
row-sum matmuls (ones_bf16 @ racc_bf16, 1 cyc/row). Diagonal-band tiles are
trimmed to their causal width (512-128r), shrinking S/PV/exp/add/mask work.
This keeps the PE array continuously busy (p-state stays at max clock).
"""

import numpy as np
import ml_dtypes
from contextlib import ExitStack
from collections import deque

import concourse.bass as bass
from concourse import bacc
import concourse.mybir as mybir
import concourse.tile as tile
from concourse.bass_utils import run_bass_kernel_spmd
from concourse.masks import make_identity

F32 = mybir.dt.float32
BF16 = mybir.dt.bfloat16
IDENT = mybir.ActivationFunctionType.Identity
EXP = mybir.ActivationFunctionType.Exp

D = 2048
T = 2048
DH = 128
B = 2
HPC = 4            # query heads per core
NCORES = 8
SCALE = 1.0 / float(np.sqrt(128.0))

_CACHE = {}


def _build_nc():
    nc = bacc.Bacc("TRN2", target_bir_lowering=False, debug=False,
                   num_devices=NCORES)

    # host-marshalled layouts: partition-major [p, block, cols]
    xt0 = nc.dram_tensor("xt0", [128, 16 * 512], BF16, kind="ExternalInput")
    xt1 = nc.dram_tensor("xt1", [128, 16 * 512], BF16, kind="ExternalInput")
    xt2 = nc.dram_tensor("xt2", [128, 16 * 512], BF16, kind="ExternalInput")
    xt3 = nc.dram_tensor("xt3", [128, 16 * 512], BF16, kind="ExternalInput")
    # wq: [p, h(4), kb(16), 128]
    wq = nc.dram_tensor("wq", [128, HPC * 16 * 128], BF16, kind="ExternalInput")
    wk = nc.dram_tensor("wk", [128, 16 * 128], BF16, kind="ExternalInput")
    wv = nc.dram_tensor("wv", [128, 16 * 128], BF16, kind="ExternalInput")
    wo = nc.dram_tensor("wo", [128, HPC * D], BF16, kind="ExternalInput")
    bqm = nc.dram_tensor("bqm", [DH, HPC], F32, kind="ExternalInput")
    bkm = nc.dram_tensor("bkm", [DH, 1], F32, kind="ExternalInput")
    bvm = nc.dram_tensor("bvm", [DH, 1], F32, kind="ExternalInput")
    part = nc.dram_tensor("part", [T, D], BF16, kind="ExternalOutput")

    with ExitStack() as ctx:
        tc = ctx.enter_context(tile.TileContext(nc))
        persist = ctx.enter_context(tc.tile_pool(name="persist", bufs=1))
        work = ctx.enter_context(tc.tile_pool(name="work", bufs=3))
        psum = ctx.enter_context(tc.tile_pool(name="psum", bufs=2, space="PSUM"))

        # ---- SBUF destinations for inputs (big consolidated tiles) ----
        wk_sb = persist.tile([128, 16 * 128], BF16, tag="wk", name="wk_sb")
        wq_sb = persist.tile([128, HPC * 16 * 128], BF16, tag="wq",
                             name="wq_sb")
        wv_sb = persist.tile([128, 16 * 128], BF16, tag="wv", name="wv_sb")
        wo_sb = persist.tile([128, HPC * D], BF16, tag="wo", name="wo_sb")
        xs_sb = [persist.tile([128, 16 * 512], BF16, tag=f"x{j}",
                              name=f"x{j}_sb") for j in range(4)]
        bq_sb = persist.tile([DH, HPC], F32, tag="bq", name="bq_sb")
        bk_sb = persist.tile([DH, 1], F32, tag="bk", name="bk_sb")
        bv_sb = persist.tile([DH, 1], F32, tag="bv", name="bv_sb")

        def wkap(kb):
            return wk_sb[:, kb * 128:(kb + 1) * 128]

        def wvap(kb):
            return wv_sb[:, kb * 128:(kb + 1) * 128]

        def wqap(h, kb):
            o = h * 2048 + kb * 128
            return wq_sb[:, o:o + 128]

        def woap(h, nsl):
            return wo_sb[:, h * 2048 + nsl.start:h * 2048 + nsl.stop]

        def xap(kb, j):
            return xs_sb[j][:, kb * 512:(kb + 1) * 512]

        # ---- input DMAs first ----
        # balanced across the three queues (~120 GB/s each) so each
        # kb-group's (wk, x0, wq) chunks land together in consumption order
        def wqg(g):
            return slice(g * 2048, (g + 1) * 2048)

        def x0c(c):
            return slice(c * 2048, (c + 1) * 2048)

        xts = [xt0, xt1, xt2, xt3]
        # round-0 prefix (K + Q0) needs only wk + x0 + wq_h0 = 3MB;
        # later heads' wq stream in behind while S phases run
        nc.sync.dma_start(out=wk_sb, in_=wk[:, :])
        nc.scalar.dma_start(out=xs_sb[0][:, 0:4096], in_=xt0[:, 0:4096])
        nc.gpsimd.dma_start(out=wq_sb[:, 0:2048], in_=wq[:, 0:2048])
        nc.sync.dma_start(out=xs_sb[0][:, 4096:8192], in_=xt0[:, 4096:8192])
        nc.gpsimd.dma_start(out=bq_sb, in_=bqm[:, :])
        nc.gpsimd.dma_start(out=bk_sb, in_=bkm[:, :])
        nc.gpsimd.dma_start(out=bv_sb, in_=bvm[:, :])
        nc.scalar.dma_start(out=wq_sb[:, 2048:4096], in_=wq[:, 2048:4096])
        nc.gpsimd.dma_start(out=wq_sb[:, 4096:6144], in_=wq[:, 4096:6144])
        nc.sync.dma_start(out=wv_sb, in_=wv[:, :])
        nc.scalar.dma_start(out=wq_sb[:, 6144:8192], in_=wq[:, 6144:8192])
        nc.gpsimd.dma_start(out=wo_sb, in_=wo[:, :])
        for j in range(1, 4):
            nc.sync.dma_start(out=xs_sb[j][:, 0:4096], in_=xts[j][:, 0:4096])
            nc.scalar.dma_start(out=xs_sb[j][:, 4096:8192],
                                in_=xts[j][:, 4096:8192])

        # ---- constants ----
        ones_bf = persist.tile([128, 128], BF16, tag="ones", name="ones_bf")
        nc.vector.memset(ones_bf, 1.0)
        ident = persist.tile([128, 128], BF16, tag="ident", name="ident")
        make_identity(nc, ident)

        # ---- persistent activations ----
        qT = [persist.tile([128, T], BF16, tag=f"qT{h}", name=f"qT{h}")
              for h in range(HPC)]
        kT = persist.tile([128, T], BF16, tag="kT", name="kT")
        v_sb = [persist.tile([128, DH], BF16, tag=f"v{t}", name=f"v{t}")
                for t in range(16)]
        oT = [persist.tile([128, T], BF16, tag=f"oT{h}", name=f"oT{h}")
              for h in range(HPC)]

        # ---- filler machinery: FIFO of (kind, fn) emitting one PE op each ----
        filler = deque()

        def pull(n):
            for _ in range(n):
                if not filler:
                    return
                filler.popleft()[1]()

        def drain(kinds=None):
            while filler and (kinds is None or filler[0][0] in kinds):
                filler.popleft()[1]()

        def push_Q(h, j):
            # next head's Q projection, pushed to the FRONT so it drains
            # during the current head's S phase (filler)
            sl = slice(512 * j, 512 * (j + 1))
            st = {}

            def mk(kb):
                def f():
                    if kb == 0:
                        st['p'] = psum.tile([128, 512], F32, tag="fill",
                                            bufs=2, name=f"qf{j}_{h}")
                    nc.tensor.matmul(out=st['p'], lhsT=wqap(h, kb),
                                     rhs=xap(kb, j),
                                     start=(kb == 0), stop=(kb == 15))
                    if kb == 15:
                        nc.scalar.activation(out=qT[h][:, sl], in_=st['p'],
                                             func=IDENT,
                                             bias=bq_sb[:, h:h + 1],
                                             scale=1.0)
                return f
            for kb in reversed(range(16)):
                filler.appendleft(('Qp', mk(kb)))

        def push_V(j):
            st = {}

            def mk(kb):
                def f():
                    if kb == 0:
                        st['p'] = psum.tile([128, 512], F32, tag="fill",
                                            bufs=2, name=f"vps{j}")
                    nc.tensor.matmul(out=st['p'], lhsT=wvap(kb),
                                     rhs=xap(kb, j),
                                     start=(kb == 0), stop=(kb == 15))
                    if kb == 15:
                        st['vt'] = work.tile([128, 512], BF16, tag="vt",
                                             bufs=2, name=f"vt{j}")
                        nc.scalar.activation(out=st['vt'], in_=st['p'],
                                             func=IDENT, bias=bv_sb[:, 0:1],
                                             scale=1.0)
                return f
            for kb in range(16):
                filler.append(('V', mk(kb)))

            def mt(sub):
                def f():
                    if sub == 0:
                        st['tp'] = psum.tile([128, 512], BF16, tag="fill",
                                             bufs=2, name=f"vtp{j}")
                    nc.tensor.transpose(st['tp'][:, sub * 128:(sub + 1) * 128],
                                        st['vt'][:, sub * 128:(sub + 1) * 128],
                                        ident)
                    nc.vector.tensor_copy(
                        out=v_sb[4 * j + sub],
                        in_=st['tp'][:, sub * 128:(sub + 1) * 128])
                return f
            for sub in range(4):
                filler.append(('V', mt(sub)))

        def push_O(j):
            # output projection for the 4 t-tiles of tq-slice j
            for sub in range(4):
                tt = 4 * j + sub
                st = {}

                def mk(n, h, tt=tt, st=st):
                    def f():
                        if h == 0:
                            if n == 0:
                                st['g'] = work.tile([128, D], BF16, tag="ostg",
                                                    bufs=2, name=f"ostg{tt}")
                            st['p'] = psum.tile([128, 512], F32, tag="fill",
                                                bufs=2, name=f"ops{tt}_{n}")
                        nsl = slice(n * 512, (n + 1) * 512)
                        nc.tensor.matmul(
                            out=st['p'],
                            lhsT=oT[h][:, tt * 128:(tt + 1) * 128],
                            rhs=woap(h, nsl),
                            start=(h == 0), stop=(h == HPC - 1))
                        if h == HPC - 1:
                            if n % 2 == 0:
                                nc.scalar.activation(out=st['g'][:, nsl],
                                                     in_=st['p'], func=IDENT,
                                                     bias=0.0, scale=1.0)
                            else:
                                nc.vector.tensor_copy(out=st['g'][:, nsl],
                                                      in_=st['p'])
                            nc.sync.dma_start(
                                out=part[tt * 128:(tt + 1) * 128, nsl],
                                in_=st['g'][:, nsl])
                    return f
                for n in range(4):
                    for h in range(HPC):
                        filler.append(('O', mk(n, h)))

        def push_PV(h, j, pts, racc):
            ntk = 4 * (j + 1)
            st = {}

            def mk(tkb):
                r = tkb - 4 * j
                off = 128 * r if r > 0 else 0

                def f():
                    if tkb == 0:
                        st['p'] = psum.tile([128, 512], F32, tag="att",
                                            bufs=3, name=f"otps{h}_{j}")
                    nc.tensor.matmul(out=st['p'][:, off:512],
                                     lhsT=v_sb[tkb], rhs=pts[tkb][:, off:512],
                                     start=(tkb == 0), stop=(tkb == ntk - 1))
                return f
            for tkb in range(ntk):
                filler.append(('PV', mk(tkb)))

            def rs():
                rsb = psum.tile([128, 512], F32, tag="att", bufs=3,
                                name=f"rsb{h}_{j}")
                nc.tensor.matmul(out=rsb, lhsT=ones_bf, rhs=racc,
                                 start=True, stop=True)
                rinv = work.tile([128, 512], F32, tag="rinv", bufs=2,
                                 name=f"rinv{h}_{j}")
                nc.vector.reciprocal_approx_fast(rinv, rsb)
                nc.vector.tensor_mul(out=oT[h][:, 512 * j:512 * (j + 1)],
                                     in0=st['p'], in1=rinv)
            filler.append(('RS', rs))

        # ================= main schedule =================
        for j in range(4):
            sl = slice(512 * j, 512 * (j + 1))

            drain()
            # --- prefix: K proj + Q proj of head 0, interleaved per kb ---
            kps = psum.tile([128, 512], F32, tag="sp", bufs=3,
                            name=f"kps{j}")
            qps0 = psum.tile([128, 512], F32, tag="sp", bufs=3,
                             name=f"qps{j}_0")
            for kb in range(16):
                nc.tensor.matmul(out=kps, lhsT=wkap(kb), rhs=xap(kb, j),
                                 start=(kb == 0), stop=(kb == 15))
                nc.tensor.matmul(out=qps0, lhsT=wqap(0, kb), rhs=xap(kb, j),
                                 start=(kb == 0), stop=(kb == 15))
            nc.scalar.activation(out=kT[:, sl], in_=kps, func=IDENT,
                                 bias=bk_sb[:, 0:1], scale=1.0)
            nc.scalar.activation(out=qT[0][:, sl], in_=qps0, func=IDENT,
                                 bias=bq_sb[:, 0:1], scale=1.0)

            # --- queue this round's filler ---
            push_V(j)
            if j > 0:
                push_O(j - 1)

            # --- attention S phases ---
            ntk = 4 * (j + 1)
            for h in range(HPC):
                if h > 0:
                    drain(kinds=('Qp',))   # qT[h] must be complete
                if h < 3:
                    push_Q(h + 1, j)
                racc = work.tile([128, 512], BF16, tag="racc", bufs=3,
                                 name=f"racc{h}_{j}")
                pts = []
                for tkb in range(ntk):
                    r = tkb - 4 * j
                    off = 128 * r if r > 0 else 0
                    sps = psum.tile([128, 512], F32, tag="sp", bufs=3,
                                    name=f"sps{h}_{j}_{tkb}")
                    nc.tensor.matmul(
                        out=sps[:, off:512],
                        lhsT=kT[:, tkb * 128:(tkb + 1) * 128],
                        rhs=qT[h][:, 512 * j + off:512 * (j + 1)],
                        start=True, stop=True)
                    pt = work.tile([128, 512], BF16, tag="pt", bufs=24,
                                   name=f"pt{h}_{j}_{tkb}")
                    nc.scalar.activation(out=pt[:, off:512],
                                         in_=sps[:, off:512],
                                         func=EXP, scale=SCALE)
                    if r >= 0:
                        nc.gpsimd.affine_select(
                            out=pt[:, off:off + 128], in_=pt[:, off:off + 128],
                            compare_op=mybir.AluOpType.is_ge,
                            fill=0.0, base=0,
                            pattern=[[1, 128]],
                            channel_multiplier=-1)
                    if tkb == 0:
                        nc.vector.tensor_copy(out=racc, in_=pt)
                    else:
                        nc.vector.tensor_add(out=racc[:, off:512],
                                             in0=racc[:, off:512],
                                             in1=pt[:, off:512])
                    pts.append(pt)
                    pull(2 if r < 2 else 1)
                push_PV(h, j, pts, racc)

        # tail: previous-head PV/rowsum + O proj of slice 3
        drain()
        push_O(3)
        drain()

    nc.compile()
    return nc


def _get_nc():
    if "nc" not in _CACHE:
        _CACHE["nc"] = _build_nc()
    return _CACHE["nc"]


def _bf16(a):
    return np.ascontiguousarray(a.astype(ml_dtypes.bfloat16))


def kernel(x, Wq, bq, Wk, bk, Wv, bv, Wo, bo, **kw):
    x = np.asarray(x, dtype=np.float32)
    Wq = np.asarray(Wq, dtype=np.float32)
    Wk = np.asarray(Wk, dtype=np.float32)
    Wv = np.asarray(Wv, dtype=np.float32)
    Wo = np.asarray(Wo, dtype=np.float32)
    bq = np.asarray(bq, dtype=np.float32)
    bk = np.asarray(bk, dtype=np.float32)
    bv = np.asarray(bv, dtype=np.float32)
    bo = np.asarray(bo, dtype=np.float32)

    nc = _get_nc()

    def pmaj(a, nblk, cols):
        # [nblk*128, cols] -> partition-major [128, nblk*cols]
        return np.ascontiguousarray(
            a.reshape(nblk, 128, cols).transpose(1, 0, 2).reshape(
                128, nblk * cols))

    xt_b = []
    for b in range(B):
        xb = x[b].T                                 # [D, T]
        xt_b.append([_bf16(pmaj(xb[:, 512 * j:512 * (j + 1)], 16, 512))
                     for j in range(4)])
    kv_cache = {}
    for kv in (0, 1):
        kv_cache[kv] = (_bf16(pmaj(Wk[:, kv * DH:(kv + 1) * DH], 16, DH)),
                        _bf16(pmaj(Wv[:, kv * DH:(kv + 1) * DH], 16, DH)))
    in_maps = []
    for c in range(NCORES):
        b = c // 4
        q = c % 4
        hs = q * HPC * DH          # column start in Wq / row start in Wo
        kv = q // 2
        # wq: [p, h, kb, 128] layout
        wq_m = np.ascontiguousarray(
            Wq[:, hs:hs + HPC * DH].reshape(16, 128, HPC, DH)
            .transpose(1, 2, 0, 3).reshape(128, HPC * 16 * DH))
        wo_m = pmaj(Wo[hs:hs + HPC * DH, :], HPC, D)
        bq_m = np.ascontiguousarray(
            bq[hs:hs + HPC * DH].reshape(HPC, DH).T)          # [128, 4]
        bk_m = np.ascontiguousarray(
            bk[kv * DH:(kv + 1) * DH].reshape(DH, 1))         # [128, 1]
        bv_m = np.ascontiguousarray(
            bv[kv * DH:(kv + 1) * DH].reshape(DH, 1))         # [128, 1]
        in_maps.append({
            "xt0": xt_b[b][0],
            "xt1": xt_b[b][1],
            "xt2": xt_b[b][2],
            "xt3": xt_b[b][3],
            "wq": _bf16(wq_m),
            "wk": kv_cache[kv][0],
            "wv": kv_cache[kv][1],
            "wo": _bf16(wo_m),
            "bqm": bq_m,
            "bkm": bk_m,
            "bvm": bv_m,
        })

    res = run_bass_kernel_spmd(nc, in_maps, list(range(NCORES)),
                               **kw.get("_run_kwargs", {}))
    if kw.get("_return_res"):
        return res
    parts = [res.results[c]["part"] for c in range(NCORES)]
    out = np.empty((B, T, D), dtype=np.float32)
    for b in range(B):
        acc = parts[4 * b].astype(np.float32)
        for q in range(1, 4):
            acc = acc + parts[4 * b + q].astype(np.float32)
        out[b] = acc + bo[None, :]
    return out
